# revision 11
# baseline (speedup 1.0000x reference)
"""Tensor-parallel llama-style attention (prefill) on 8 TRN2 NeuronCores.

Sharding: tensor-parallel over heads. Core c holds q-heads [4c, 4c+4),
kv-head c, the matching rows of wq/wk/wv, and columns [512c, 512c+512) of
wo. Each core computes a full-size partial of the output projection;
partials are summed on the host (the "all-reduce after wo").

v2: fully-fused single-pipeline design.
  - Everything bf16 (fp32 PSUM accumulation): halves DMA/SBUF traffic,
    enables FWL fast weight loads. fp8 fails the 2e-2 rel-err budget
    (measured 3.3e-2..6.6e-2 in simulation); bf16 lands ~4e-3.
  - The three phases (P1 qkv-projection+RoPE, A attention, W out-proj)
    are interleaved at instruction-emission granularity in a 10-slot
    software pipeline: slot T runs P1(tb=T), A(qb=T-1), W(qb=T-2)
    (shifted by 4 per batch). The scalar-engine exp (~650ns per
    [128,512] tile, dtype-independent) hides behind P1/W matmuls, and
    the PE never idles long enough for HAM to re-throttle the clock.
  - Causal suffix trick: for diagonal-band k-tiles only the unmasked
    column suffix [q0:512] of scores/exp/softmax-sum/AV is computed
    (the skipped prefix is exactly-0 after exp anyway).
  - Softmax row-sums via an all-ones [128,128] stationary matmul ->
    sums replicated on all 128 partitions -> reciprocal_approx_fast
    (single DVE op) -> broadcast-free normalize. No gpsimd broadcast,
    no 8-cycle-per-element DVE reciprocal.
  - PSUM budget (8 banks): P1 2 (three passes of 2 outputs over the
    k-dim, x streamed 3x), A 5 (scores x2, o x2, sum x1), W/transpose 1.
    Pools are stage-private so interleaved emission cannot deadlock on
    bank WAR semaphores.
  - W-phase PSUM->SBUF copies run on the otherwise-idle gpsimd (Pool)
    engine, with the out DMA triggered from the same queue.
"""

import math
import os
import sys

sys.path.insert(0, "/opt/trn_rl_repo")

import numpy as np
import ml_dtypes

import concourse.bacc as bacc
import concourse.tile as tile
import concourse.mybir as mybir
from concourse import masks
from concourse.bass_utils import run_bass_kernel_spmd

B, S, DIM = 2, 2048, 4096
TOK = B * S
NH, NKV, HD = 32, 8, 128
NCORES = 8
HQ = NH // NCORES            # 4 query heads per core
NJ = HQ + 2                  # 6 projection outputs: 4 Q heads, K, V
SCALE = 1.0 / math.sqrt(HD)
F32 = mybir.dt.float32
BF16 = mybir.dt.bfloat16
EXP = mybir.ActivationFunctionType.Exp

QB = 4          # q-blocks per batch (512 queries each)
QW = S // QB    # 512
KT = S // 128   # 16 k-tiles per batch
BF = ml_dtypes.bfloat16


def _merge(lists):
    """Proportionally interleave several unit lists into one stream."""
    items = []
    for li, lst in enumerate(lists):
        n = len(lst)
        for i, u in enumerate(lst):
            items.append(((i + 0.5) / n, li, i, u))
    items.sort(key=lambda t: (t[0], t[1]))
    return [t[3] for t in items]


def _build(causal: bool):
    nc = bacc.Bacc("TRN2", target_bir_lowering=False, debug=False)

    xT_d = nc.dram_tensor("xT", [DIM, TOK], BF16, kind="ExternalInput")
    w_d = nc.dram_tensor("wqkvT", [DIM, NJ * HD], BF16, kind="ExternalInput")
    wo_d = nc.dram_tensor("woT", [HQ * HD, DIM], BF16, kind="ExternalInput")
    cos_d = nc.dram_tensor("cosT", [HD, S], BF16, kind="ExternalInput")
    sin_d = nc.dram_tensor("sinTs", [HD, S], BF16, kind="ExternalInput")
    if causal:
        mask_d = nc.dram_tensor("maskTd", [4, 128, QW], F32,
                                kind="ExternalInput")
    else:
        mask_d = nc.dram_tensor("maskTd", [QB, KT, 128, QW], F32,
                                kind="ExternalInput")
    out_d = nc.dram_tensor("out_part", [TOK, DIM], BF16,
                           kind="ExternalOutput")

    xT = xT_d.ap().rearrange("(kt p) t -> p kt t", p=128)      # [128, 32, TOK]
    w_ap = w_d.ap().rearrange("(kt p) j -> p kt j", p=128)     # [128, 32, 768]
    wo_ap = wo_d.ap().rearrange("(dt p) m -> p dt m", p=128)   # [128, 4, DIM]
    out_v = out_d.ap().rearrange("(g p) m -> p g m", p=128)    # [128, 32, DIM]

    PASSES = [(0, 1), (2, 3), (4, 5)]   # j-pairs: Q0Q1, Q2Q3, KV

    with tile.TileContext(nc) as tc:
        with (
            tc.tile_pool(name="const", bufs=1) as cpool,
            tc.tile_pool(name="kv", bufs=2) as kvp,
            tc.tile_pool(name="qs", bufs=2) as qsp,
            tc.tile_pool(name="at", bufs=3) as atp,
            tc.tile_pool(name="xt", bufs=3) as xtp,
            tc.tile_pool(name="rp", bufs=1) as rpp,
            tc.tile_pool(name="vb", bufs=2) as vbp,
            tc.tile_pool(name="pt", bufs=3) as ptp,
            tc.tile_pool(name="rc", bufs=2) as rcp,
            tc.tile_pool(name="ac", bufs=2) as acp,
            tc.tile_pool(name="ob", bufs=2) as obp,
            tc.tile_pool(name="mp", bufs=2) as mpp,
            tc.tile_pool(name="p1ps", bufs=2, space="PSUM") as p1ps,
            tc.tile_pool(name="aps", bufs=1, space="PSUM") as aps,
            tc.tile_pool(name="wps", bufs=1, space="PSUM") as wps,
        ):
            # ---------------- constants ----------------
            wqkv = cpool.tile([128, 32, NJ * HD], BF16)
            for kc in range(4):
                for p in range(3):
                    nc.scalar.dma_start(
                        wqkv[:, kc * 8:(kc + 1) * 8, p * 256:(p + 1) * 256],
                        w_ap[:, kc * 8:(kc + 1) * 8, p * 256:(p + 1) * 256])
            cos_s = cpool.tile([128, S], BF16)
            sin_s = cpool.tile([128, S], BF16)
            nc.scalar.dma_start(cos_s[:], cos_d.ap())
            nc.scalar.dma_start(sin_s[:], sin_d.ap())
            ident = cpool.tile([128, 128], BF16)
            masks.make_identity(nc, ident[:])
            ones_f = cpool.tile([128, 128], F32)
            nc.vector.memset(ones_f[:], 1.0)
            ones32 = cpool.tile([128, 128], mybir.dt.float32r)
            nc.vector.tensor_copy(ones32[:], ones_f[:])
            if causal:
                mask_s = cpool.tile([128, 4, QW], F32)
                nc.scalar.dma_start(
                    mask_s[:], mask_d.ap().rearrange("j p q -> p j q"))
            wo_s = cpool.tile([128, 4, DIM], BF16)

            # shared tile handles created lazily inside thunks
            kT_t = {}     # b -> [128, S] bf16
            v_t = {}      # b -> [128, KT, HD] bf16
            q_t = {}      # (b, qb) -> [128, HQ, QW] bf16
            att_t = {}    # (b, qb) -> [128, HQ, QW] bf16
            m_t = {}      # (b, qb) -> [128, KT, QW] f32   (non-causal only)

            # ---------------- P1: projections + RoPE ----------------
            def p1_units(b):
                units = [[] for _ in range(4)]   # per tb
                for tb in range(4):
                    c0 = b * S + tb * 512
                    sl = slice(tb * 512, tb * 512 + 512)
                    u = units[tb]

                    if tb == 0:
                        def alloc_kv(b=b):
                            kT_t[b] = kvp.tile([128, S], BF16, tag="k",
                                               name="kT")
                            v_t[b] = kvp.tile([128, KT, HD], BF16, tag="v",
                                              name="vS")
                        u.append(alloc_kv)

                    # Slot 0 has no A stage running, so its P1 can borrow the
                    # (idle) A-pool PSUM banks: one 6-wide pass, x read once.
                    single = (b == 0 and tb == 0)
                    passes = [(0, 1, 2, 3, 4, 5)] if single else PASSES
                    for pi, pjs in enumerate(passes):
                        ps_tiles = {}

                        def palloc(ps_tiles=ps_tiles, single=single):
                            ps_tiles[0] = p1ps.tile([128, 512], F32, tag="ps",
                                                    name="psA")
                            ps_tiles[1] = p1ps.tile([128, 512], F32, tag="ps",
                                                    name="psB")
                            if single:
                                ps_tiles[2] = aps.tile([128, 512], F32,
                                                       tag="sps", bufs=2,
                                                       name="psC")
                                ps_tiles[3] = aps.tile([128, 512], F32,
                                                       tag="sps", bufs=2,
                                                       name="psD")
                                ps_tiles[4] = aps.tile([128, 512], F32,
                                                       tag="ops", bufs=2,
                                                       name="psE")
                                ps_tiles[5] = aps.tile([128, 512], F32,
                                                       tag="ops", bufs=2,
                                                       name="psF")
                        u.append(palloc)

                        xt_tiles = {}

                        def xload(ks, c0=c0, xt_tiles=xt_tiles):
                            xt_tiles[ks] = xtp.tile([128, 8, 512], BF16,
                                                    tag="xt", name="xt_c")
                            nc.sync.dma_start(
                                xt_tiles[ks],
                                xT[:, ks * 8:(ks + 1) * 8, c0:c0 + 512])

                        # prefetch depth 3 (xt pool has 3 bufs): two chunks
                        # in flight while one is being consumed
                        u.append(lambda f=xload: f(0))
                        u.append(lambda f=xload: f(1))
                        for ks in range(4):
                            if ks + 2 < 4:
                                u.append(lambda f=xload, ks=ks: f(ks + 2))

                            def mmh(ks=ks, pjs=pjs, ps_tiles=ps_tiles,
                                    xt_tiles=xt_tiles, lo=True):
                                xt_c = xt_tiles[ks]
                                for k in range(0, 4) if lo else range(4, 8):
                                    for jj, j in enumerate(pjs):
                                        nc.tensor.matmul(
                                            ps_tiles[jj][:],
                                            wqkv[:, ks * 8 + k,
                                                 j * HD:(j + 1) * HD],
                                            xt_c[:, k, :],
                                            start=(ks == 0 and k == 0),
                                            stop=(ks == 3 and k == 7))
                            u.append(lambda f=mmh: f(lo=True))
                            u.append(lambda f=mmh: f(lo=False))

                        # pass finalize: RoPE / V transpose
                        for jj, j in enumerate(pjs):
                            if j < HQ + 1:
                                def rope(j=j, jj=jj, b=b, tb=tb, sl=sl,
                                         ps_tiles=ps_tiles):
                                    ps = ps_tiles[jj]
                                    tmp = rpp.tile([128, 512], F32, tag="tmp")
                                    nc.vector.tensor_mul(
                                        tmp[0:64, :], ps[64:128, :],
                                        sin_s[0:64, sl])
                                    nc.vector.tensor_mul(
                                        tmp[64:128, :], ps[0:64, :],
                                        sin_s[64:128, sl])
                                    t2 = rpp.tile([128, 512], F32, tag="t2")
                                    nc.vector.tensor_mul(
                                        t2[:], ps[:], cos_s[:, sl])
                                    if j < HQ:
                                        if (b, tb) not in q_t:
                                            q_t[(b, tb)] = qsp.tile(
                                                [128, HQ, QW], BF16, tag="q",
                                                name="qS")
                                        nc.vector.tensor_add(
                                            q_t[(b, tb)][:, j, :],
                                            t2[:], tmp[:])
                                    else:
                                        nc.vector.tensor_add(
                                            kT_t[b][:, sl], t2[:], tmp[:])
                                u.append(rope)
                            else:
                                def vfin(b=b, tb=tb, ps_tiles=ps_tiles,
                                         jj=jj):
                                    ps = ps_tiles[jj]
                                    v_sb = vbp.tile([128, 512], BF16,
                                                    tag="vsb")
                                    nc.scalar.copy(v_sb[:], ps[:])
                                    tp = wps.tile([128, 4, 128], BF16,
                                                  tag="blk", name="tpV")
                                    for i in range(4):
                                        nc.tensor.transpose(
                                            tp[:, i, :],
                                            v_sb[:, i * 128:(i + 1) * 128],
                                            ident[:])
                                    nc.vector.tensor_copy(
                                        v_t[b][:, tb * 4:tb * 4 + 4, :],
                                        tp[:])
                                u.append(vfin)
                return units

            # ---------------- A: attention ----------------
            def a_units(b, qb):
                u = []
                kmax = 4 * (qb + 1) if causal else KT

                if not causal:
                    def mload(b=b, qb=qb):
                        m_t[(b, qb)] = mpp.tile([128, KT, QW], F32, tag="m",
                                                name="mQ")
                        nc.scalar.dma_start(
                            m_t[(b, qb)],
                            mask_d.ap()[qb].rearrange("kt p q -> p kt q"))
                    # emitted one slot early by the caller
                    pre = [mload]
                else:
                    pre = []

                for h in range(HQ):
                    st = {}

                    def q0_of(kt):
                        if causal and kt >= 4 * qb:
                            return (kt - 4 * qb) * 128
                        return 0

                    def ua(kt, h=h, st=st, b=b, qb=qb):
                        q0 = q0_of(kt)
                        s_ps = aps.tile([128, 512], F32, tag="sps", bufs=2,
                                        name="sPS")
                        nc.tensor.matmul(
                            s_ps[:, q0:],
                            kT_t[b][:, kt * 128:(kt + 1) * 128],
                            q_t[(b, qb)][:, h, q0:],
                            start=True, stop=True)
                        if causal:
                            if kt >= 4 * qb:
                                j = kt - 4 * qb
                                nc.vector.tensor_add(
                                    s_ps[:, q0:], s_ps[:, q0:],
                                    mask_s[:, j, q0:])
                        else:
                            nc.vector.tensor_add(
                                s_ps[:, q0:], s_ps[:, q0:],
                                m_t[(b, qb)][:, kt, :])
                        pT = ptp.tile([128, 512], BF16, tag="pt", name="pT")
                        nc.scalar.activation(pT[:, q0:], s_ps[:, q0:], EXP,
                                             bias=0.0, scale=SCALE)
                        st[kt] = pT

                    def ub(kt, h=h, st=st, b=b, qb=qb, kmax=kmax):
                        q0 = q0_of(kt)
                        if kt == 0:
                            st["o"] = aps.tile([128, 512], F32, tag="ops",
                                               bufs=2, name="oPS")
                            # exp-sum accumulator; DVE-only (gpsimd ops are
                            # ~1.2us each and the serial chain stalls heads)
                            st["acc"] = acp.tile([128, 512],
                                                 mybir.dt.float32r,
                                                 tag="aE", name="accS")
                        pT = st.pop(kt)
                        if kt == 0:
                            nc.vector.tensor_copy(st["acc"][:], pT[:])
                        else:
                            nc.vector.tensor_add(st["acc"][:, q0:],
                                                 st["acc"][:, q0:],
                                                 pT[:, q0:])
                        nc.tensor.matmul(
                            st["o"][:, q0:], v_t[b][:, kt, :], pT[:, q0:],
                            start=(kt == 0), stop=(kt == kmax - 1))

                    def fin(h=h, st=st, b=b, qb=qb):
                        sum_ps = aps.tile([128, 512], F32, tag="sum",
                                          bufs=1, name="sumPS")
                        nc.tensor.matmul(sum_ps[:], ones32[:], st["acc"][:],
                                         start=True, stop=True)
                        r_sb = rcp.tile([128, 512], F32, tag="r", name="rS")
                        nc.vector.reciprocal_approx_fast(r_sb[:], sum_ps[:])
                        if (b, qb) not in att_t:
                            att_t[(b, qb)] = atp.tile([128, HQ, QW], BF16,
                                                      tag="a", name="attS")
                        nc.vector.tensor_mul(
                            att_t[(b, qb)][:, h, :], st["o"][:], r_sb[:])

                    u.append(lambda f=ua: f(0))
                    for kt in range(1, kmax):
                        u.append(lambda f=ua, kt=kt: f(kt))
                        u.append(lambda f=ub, kt=kt: f(kt - 1))
                    u.append(lambda f=ub, kmax=kmax: f(kmax - 1))
                    u.append(fin)
                return pre, u

            # ---------------- W: output projection ----------------
            def w_units(b, qb, mbs, borrow=False, dve_copy=False):
                u = []
                g0 = b * (S // 128) + qb * 4
                for mb in mbs:
                    osb = {}
                    for tt in range(4):
                        def wg(mb=mb, tt=tt, osb=osb, b=b, qb=qb, g0=g0,
                               dve_copy=dve_copy):
                            if tt == 0:
                                osb[0] = obp.tile([128, 4, 512], BF16,
                                                  tag="o", name="oSB")
                            # in the A-free tail slots, borrow the idle
                            # A-pool PSUM banks so W groups pipeline instead
                            # of ping-ponging PE<->ACT on one bank
                            if borrow:
                                r = (mb * 4 + tt) % 5
                                if r < 2:
                                    ps_w = aps.tile([128, 512], F32,
                                                    tag="sps", bufs=2,
                                                    name="wPS")
                                elif r < 4:
                                    ps_w = aps.tile([128, 512], F32,
                                                    tag="ops", bufs=2,
                                                    name="wPS")
                                else:
                                    ps_w = wps.tile([128, 512], F32,
                                                    tag="blk", name="wPS")
                            else:
                                ps_w = wps.tile([128, 512], F32, tag="blk",
                                                name="wPS")
                            att = att_t[(b, qb)]
                            for d4 in range(HQ):
                                nc.tensor.matmul(
                                    ps_w[:],
                                    att[:, d4, tt * 128:(tt + 1) * 128],
                                    wo_s[:, d4, mb * 512:(mb + 1) * 512],
                                    start=(d4 == 0), stop=(d4 == HQ - 1))
                            # gpsimd/Pool cannot read PSUM; ACT takes the W
                            # copies by default (DVE owns the softmax-sum
                            # chain), except in slots where exp saturates
                            # ACT. The out DMA triggers from the copying
                            # engine's own queue so nothing blocks on a wait.
                            if dve_copy:
                                nc.vector.tensor_copy(osb[0][:, tt, :],
                                                      ps_w[:])
                            else:
                                nc.scalar.copy(osb[0][:, tt, :], ps_w[:])
                            if tt == 3:
                                # DVE can't trigger DMAs; gpsimd (idle) can
                                deng = nc.gpsimd if dve_copy else nc.scalar
                                deng.dma_start(
                                    out_v[:, g0:g0 + 4,
                                          mb * 512:(mb + 1) * 512],
                                    osb[0])
                        u.append(wg)
                return u

            # ---------------- slot assembly ----------------
            def wo_load(half):
                nc.scalar.dma_start(
                    wo_s[:, half * 2:half * 2 + 2, :],
                    wo_ap[:, half * 2:half * 2 + 2, :])

            if causal:
                nslots = 10
                slots = [[] for _ in range(nslots)]
                for b in range(B):
                    pu = p1_units(b)
                    for tb in range(4):
                        slots[4 * b + tb].append(pu[tb])
                    for qb in range(QB):
                        pre, au = a_units(b, qb)
                        slots[4 * b + qb + 1].append(au)
                        # W split across two slots to keep the pipeline
                        # tail dense (slot 9 would otherwise run alone)
                        t1 = 4 * b + qb + 2
                        t2 = min(t1 + 1, nslots - 1)
                        if t1 == t2:
                            slots[t1].append(
                                w_units(b, qb, range(8), borrow=True))
                        else:
                            # slot 8 pairs the heaviest A stage (qb=3 of
                            # batch 1) with W work: exp saturates ACT there,
                            # so that slot's W copies go to DVE instead
                            slots[t1].append(
                                w_units(b, qb, range(4),
                                        dve_copy=(t1 == nslots - 2)))
                            slots[t2].append(
                                w_units(b, qb, range(4, 8),
                                        borrow=(t2 == nslots - 1)))
            else:
                nslots = 14
                slots = [[] for _ in range(nslots)]
                for b in range(B):
                    pu = p1_units(b)
                    for tb in range(4):
                        slots[4 * b + tb].append(pu[tb])
                    for qb in range(QB):
                        pre, au = a_units(b, qb)
                        slots[4 * b + qb + 4].append(pre)
                        slots[4 * b + qb + 5].append(au)
                        slots[4 * b + qb + 6].append(w_units(b, qb, range(8)))
            slots[1].append([lambda: wo_load(0), lambda: wo_load(1)])

            for T in range(nslots):
                for unit in _merge(slots[T]):
                    unit()

    nc.compile()
    return nc


_CACHE = {}
LAST_EXEC_NS = None


def _get_nc(causal: bool):
    if causal not in _CACHE:
        _CACHE[causal] = _build(causal)
    return _CACHE[causal]


def _host_prep(x, wq, wk, wv, wo, freqs_cos, freqs_sin, mask):
    perm = np.concatenate([np.arange(0, HD, 2), np.arange(1, HD, 2)])
    wq_p = wq.reshape(NH, HD, DIM)[:, perm, :].reshape(NH * HD, DIM)
    wk_p = wk.reshape(NKV, HD, DIM)[:, perm, :].reshape(NKV * HD, DIM)

    xT = np.ascontiguousarray(x.reshape(TOK, DIM).T.astype(BF))

    cos = freqs_cos.T                     # [64, S]
    sin = freqs_sin.T
    cosT = np.ascontiguousarray(np.concatenate([cos, cos], 0).astype(BF))
    sinTs = np.ascontiguousarray(np.concatenate([-sin, sin], 0).astype(BF))

    ref_mask = np.triu(np.full((S, S), -1e9, dtype=np.float32), k=1)
    causal = np.array_equal(mask, ref_mask)

    maskT = np.ascontiguousarray(mask.T) / np.float32(SCALE)   # [k, q]
    if causal:
        maskTd = np.empty((4, 128, QW), dtype=np.float32)
        for j in range(4):
            maskTd[j] = maskT[j * 128:(j + 1) * 128, 0:QW]
    else:
        maskTd = np.empty((QB, KT, 128, QW), dtype=np.float32)
        for qb in range(QB):
            for kt in range(KT):
                maskTd[qb, kt] = maskT[kt * 128:(kt + 1) * 128,
                                       qb * QW:(qb + 1) * QW]

    in_maps = []
    for c in range(NCORES):
        wqT = wq_p[c * HQ * HD:(c + 1) * HQ * HD, :].T          # [DIM, 512]
        wkT = wk_p[c * HD:(c + 1) * HD, :].T                    # [DIM, 128]
        wvT = wv[c * HD:(c + 1) * HD, :].T                      # [DIM, 128]
        wqkvT = np.ascontiguousarray(
            np.concatenate([wqT, wkT, wvT], 1).astype(BF))
        woT = np.ascontiguousarray(
            wo[:, c * HQ * HD:(c + 1) * HQ * HD].T.astype(BF))
        in_maps.append({
            "xT": xT, "wqkvT": wqkvT, "woT": woT,
            "cosT": cosT, "sinTs": sinTs, "maskTd": maskTd,
        })
    return causal, in_maps


def kernel(x, wq, wk, wv, wo, freqs_cos, freqs_sin, mask, start_pos):
    global LAST_EXEC_NS
    causal, in_maps = _host_prep(
        np.asarray(x, np.float32), np.asarray(wq, np.float32),
        np.asarray(wk, np.float32), np.asarray(wv, np.float32),
        np.asarray(wo, np.float32), np.asarray(freqs_cos, np.float32),
        np.asarray(freqs_sin, np.float32), np.asarray(mask, np.float32))

    nc = _get_nc(causal)
    res = run_bass_kernel_spmd(nc, in_maps, core_ids=list(range(NCORES)))
    LAST_EXEC_NS = res.exec_time_ns

    acc = res.results[0]["out_part"].astype(np.float64)
    for c in range(1, NCORES):
        acc += res.results[c]["out_part"].astype(np.float64)
    return acc.astype(np.float32).reshape(B, S, DIM)


def _ref_core(xT, wqkvT, woT, cosT, sinTs, mask):
    """Numpy reference for ONE core's partial output (fp32 math, permuted
    rope basis exactly as the device computes it)."""
    xf = xT.astype(np.float32).T                 # [TOK, DIM]
    proj = xf @ wqkvT.astype(np.float32)         # [TOK, 768]
    cos = cosT.astype(np.float32)                # [128, S]
    sin = sinTs.astype(np.float32)               # [128, S] rows 0:64 = -sin
    out = np.zeros((TOK, DIM), np.float64)

    def rope(t):
        # t: [S, 128] in permuted basis; swap halves along feature dim
        sw = np.concatenate([t[:, 64:], t[:, :64]], 1)
        return t * cos.T + sw * sin.T

    for b in range(B):
        tok = slice(b * S, (b + 1) * S)
        pb = proj[tok]
        k = rope(pb[:, HQ * HD:(HQ + 1) * HD])   # [S, 128]
        v = pb[:, (HQ + 1) * HD:]                # [S, 128]
        ctx = np.empty((S, HQ * HD), np.float32)
        for h in range(HQ):
            q = rope(pb[:, h * HD:(h + 1) * HD])
            s_ = (q @ k.T) * SCALE + mask
            s_ = s_ - s_.max(-1, keepdims=True)
            p = np.exp(s_)
            p /= p.sum(-1, keepdims=True)
            ctx[:, h * HD:(h + 1) * HD] = p @ v
        out[tok] = ctx @ woT.astype(np.float32)
    return out


if __name__ == "__main__":
    mode = os.environ.get("MODE", "build")
    causal_env = os.environ.get("CAUSAL", "1") == "1"
    if mode == "build":
        nc = _build(causal_env)
        print("build OK")
    elif mode == "sim":
        from concourse.bass_interp import CoreSim
        rng = np.random.default_rng(0)
        x = rng.standard_normal((B, S, DIM), dtype=np.float32)
        wq = rng.standard_normal((DIM, DIM), dtype=np.float32) * 0.02
        wk = rng.standard_normal((NKV * HD, DIM), dtype=np.float32) * 0.02
        wv = rng.standard_normal((NKV * HD, DIM), dtype=np.float32) * 0.02
        wo = rng.standard_normal((DIM, DIM), dtype=np.float32) * 0.02
        inv = 1.0 / (10000.0 ** (np.arange(0, HD, 2, dtype=np.float32) / HD))
        t = np.arange(S, dtype=np.float32)
        fr = np.outer(t, inv)
        fc, fs = np.cos(fr).astype(np.float32), np.sin(fr).astype(np.float32)
        if causal_env:
            mask = np.triu(np.full((S, S), -1e9, dtype=np.float32), k=1)
        else:
            mask = (rng.standard_normal((S, S)) * 0.5).astype(np.float32)
        causal, in_maps = _host_prep(x, wq, wk, wv, wo, fc, fs, mask)
        assert causal == causal_env, (causal, causal_env)
        nc = _get_nc(causal)
        print("build OK, running CoreSim...")
        sim = CoreSim(nc, require_finite=False, require_nnan=False)
        im = in_maps[0]
        for name, val in im.items():
            sim.tensor(name)[:] = val
        sim.simulate()
        got = np.asarray(sim.tensor("out_part")).astype(np.float64)
        want = _ref_core(im["xT"], im["wqkvT"], im["woT"], im["cosT"],
                         im["sinTs"], mask)
        sc = np.abs(want).max()
        err = np.abs(got - want).max()
        print(f"core0 partial: absmax err {err:.5f}, scale {sc:.3f}, "
              f"rel {err / sc:.3e}")
